# revision 27
# baseline (speedup 1.0000x reference)
"""BlockDecay (RetNet-style chunkwise linear attention with per-feature decay)
Trainium2 Bass kernel, batch-parallel over 8 NeuronCores.

Math (per batch): out[t] = sum_r q[t,r] * S_t[r,:],
  S_t[r,d] = sum_{s<=t} gamma_r^{t-s} k[s,r] h[s,d]
computed chunkwise with C=128 using the standard factorization with
CENTERED decay scaling so fp16 operands stay in range (gamma^{+-64}):
  qsT[r,i] = q * gamma^{(i%128)-64}      (AT rhs, OT-inter rhs)
  ksT[r,j] = k * gamma^{64-(j%128)}      (AT lhsT)
  k2n[j,r] = k * gamma^{192-(j%128)}     (KP lhsT, block-local layout)
  hn [j,d] = h                           (KP rhs, OT-intra lhsT)
  A^T[j,i] = ksT.T qsT ; Am = A^T o tri  (mask)
  KP[r,d]  = k2n.T hn                    (psum)
  S_{m+1}  = gamma^128 * S_m + KP_m      (DVE fp16 chain, KP read from PSUM)
  OT[d,i]  = hn.T Am + S_m.T' qsT        (PSUM fp32, fp16 evac)

Engine split (per 4-block group): PE 4 KP + 4 AT + 8 OT matmuls (all fp16,
N=128); ACT scaled-copies AT psum -> SBUF fp16 at x2^-12 (the unmasked
upper-triangle garbage reaches ~1e8 and would inf out in fp16; after /4096
everything fits) and evacuates OT; GPSIMD applies the mask by multiplying
with tri4 whose lower triangle holds 4096.0 (exact power-of-2 undo); DVE
runs the serial S chain (the pacer, ~380ns/step x32) plus the last two
groups' masks/evacs after its chain drains.

Scheduling (hard-won, see traces): the Tile scheduler emits ONE static
in-order stream per engine from its cost model, which does not know the
~1-2us DMA completion-sem latency, and the PE HAM clock-gate halves the
clock after any ~3.4us idle lapse.  Countermeasures: (1) KP matmuls are
pinned with high_priority so the static PE order can never starve the
chain; (2) the input is ONE dram tensor IN [128,4W] whose column order
interleaves KH_g=[k2n|hn] one slot ahead of QK_g=[ksT|qsT], DMA'd in 12
fine pieces alternating across both HWDGE rings (sync+scalar) so the
chain feed rides ~2 groups ahead of the AT path; (3) no-dep dummy
matmuls (dedicated PSUM bank) bridge early data stalls so HAM stays at
8/8; (4) AT trails KP by 2 iterations, OT by 4, OT-evac by 5, so no
engine FIFO head-blocks; (5) the last OT group borrows the retired
dummy-matmul PSUM bank and its evac/out-DMA is split in halves to
shorten the tail.  Output otT fp16, host transposes/casts back.
Measured: 35.3-36.1us traced, +-1.5us run variance (baseline fp32
kernel: 54-65us traced), rel err 1.14e-3 vs the 2e-2 gate.
"""
import os
import sys
import numpy as np

for _p in ("/root/.axon_site", "/root/.axon_site/_ro/trn_rl_repo",
           "/root/.axon_site/_ro/pypackages"):
    if _p not in sys.path and os.path.isdir(_p):
        sys.path.append(_p)

B, W, R, D = 8, 4096, 128, 128
NBLK = W // 128          # 32 blocks of 128
NGRP = NBLK // 4         # 8 groups of 4 blocks
GCOL = 4 * 512           # IN columns per group
LAG = 4                  # groups between KP emission and OT matmuls
ATLAG = 2                # AT matmuls trail KP by this many iterations
DVE_MASK = 2             # last N groups: mask on DVE (its chain is done)
# IN column layout, units of 1024 cols: KH_g = [k2n_g|hn_g] shipped ~2-3
# groups ahead of QK_g = [ksT_g|qsT_g] so the serial S-chain (the pacer)
# is never data-gated.
KH_POS = [0, 1, 2, 4, 6, 8, 10, 12]
QK_POS = [3, 5, 7, 9, 11, 13, 14, 15]

_PROG = {}


def _patched_tc(nc):
    """TileContext with a cheap exit: per-sem single-wait drains on sync,
    one barrier, then sem clears for idempotent re-execution."""
    import concourse.tile as tile
    import concourse.tile_sem_assignment as tsa
    from concourse.tile import ScopedClock

    class PatchedTileContext(tile.TileContext):
        def _drain_and_barrier(self, tick_clock, wait_clock):
            gc = tick_clock.global_clock
            n = tsa.N_PROCS
            nc = self.nc
            for p in range(n):
                ticks = gc[p]
                if ticks <= 0:
                    continue
                d = nc.sync.drain()
                wait_clock.add_sem_waits(
                    d.ins,
                    ScopedClock({None: tsa.VectorClock(
                        [ticks if q == p else 0 for q in range(n)])}),
                )
            nc.all_engine_barrier()
            assert self.sems is not None
            popped = nc._tile_sem_poison_stack.pop()
            assert popped is self._sem_poison
            nc.clear_and_free_semaphores(list(self.sems.allocated().values()))

    return PatchedTileContext(nc)


def _split_multi_waits(nc, limit=1):
    """Hoist extra sync-waits onto injected same-engine NoOps."""
    import concourse.mybir as mybir
    n_new = 0
    for fn in nc.m.functions:
        for bb in fn.blocks:
            out = []
            changed = False
            for inst in bb.instructions:
                si = getattr(inst, "sync_info", None)
                waits = list(si.on_wait) if si is not None and si.on_wait else []
                if len(waits) > limit:
                    for w in waits[:-limit]:
                        nop = mybir.InstNoOp(
                            name=f"I-wsplit-{n_new}",
                            engine=inst.engine,
                            sync_info=mybir.SyncInfo(on_wait=[w], on_update=[]),
                        )
                        n_new += 1
                        out.append(nop)
                    si.on_wait = waits[-limit:]
                    changed = True
                out.append(inst)
            if changed:
                bb.instructions = out
    return n_new


def _build_program():
    key = "v20_fp16"
    if key in _PROG:
        return _PROG[key]
    import concourse.bass as bass
    import concourse.mybir as mybir

    F32 = mybir.dt.float32
    F16 = mybir.dt.float16

    nc = bass.Bass()
    IN = nc.declare_dram_parameter("IN", [128, 4 * W], F16, isOutput=False)
    g128 = nc.declare_dram_parameter("g128", [128, 1], F32, isOutput=False)
    tri4 = nc.declare_dram_parameter("tri4", [128, 512], F32, isOutput=False)
    otT = nc.declare_dram_parameter("otT", [128, W], F16, isOutput=True)

    mm = nc.tensor.matmul

    def in_sl(sb, m, which):  # which: 0=k2n 1=hn 2=ksT 3=qsT
        g, q = m // 4, m % 4
        if which < 2:   # KH units interleave [k2n_b|hn_b] per block
            c = KH_POS[g] * 1024 + q * 256 + which * 128
        else:
            c = QK_POS[g] * 1024 + (which - 2) * 512 + q * 128
        return sb[:, c:c + 128]

    with _patched_tc(nc) as tc:
        with tc.tile_pool(name="big", bufs=1) as big, \
             tc.tile_pool(name="small", bufs=1) as small, \
             tc.tile_pool(name="st", bufs=33) as stp, \
             tc.tile_pool(name="ats", bufs=4) as atsp, \
             tc.tile_pool(name="amp", bufs=8) as amp, \
             tc.tile_pool(name="ps_at", bufs=2, space="PSUM") as ps_at, \
             tc.tile_pool(name="ps_ot", bufs=2, space="PSUM") as ps_ot, \
             tc.tile_pool(name="ps_kp", bufs=3, space="PSUM") as ps_kp, \
             tc.tile_pool(name="ps_w", bufs=1, space="PSUM") as ps_w:

            IN_sb = big.tile([128, 4 * W], F16, tag="IN")
            otT_sb = big.tile([128, W], F16, tag="otT")
            tri4_sb = small.tile([128, 512], F32, tag="tri4")
            g128_sb = small.tile([128, 1], F32, tag="g128")

            # PE warm-up: back-to-back dummy fp16 matmuls (ping-pong PSUM
            # banks) fill the DMA-wait window and flip HAM to 8/8.
            wz = small.tile([128, 512], F16, tag="wz")
            nc.gpsimd.memset(wz[:], 0.0)
            for _ in range(5):
                wp = ps_w.tile([128, 512], F32, tag="w")
                mm(wp[:], wz[:, :128], wz[:], start=True, stop=True)

            # KH0 first (gates the first chain step), then g128, then fine
            # pieces alternating across both HWDGE rings (sync + scalar):
            # interleaved KH/QK column order keeps the chain feed ~2 groups
            # ahead of the AT path at the stream's JIT pace.  (Finer 16-way
            # per-unit pieces and a KH/QK ring split were both measured
            # WORSE - per-piece issue+completion overheads dominate.)
            # block-0 feed (64KB) lands first: chain step 0 starts ~0.5us
            # sooner; then g128, the rest of KH0, and the regular pieces
            nc.sync.dma_start(IN_sb[:, 0:256], IN[:, 0:256])
            nc.sync.dma_start(g128_sb[:], g128[:])
            nc.sync.dma_start(IN_sb[:, 256:1024], IN[:, 256:1024])
            pieces = [(1, 2), (2, 3), (3, 4), (4, 5), (5, 6),
                      (6, 8), (8, 10), (10, 12), (12, 14), (14, 15), (15, 16)]
            for n, (lo, hi) in enumerate(pieces):
                s = slice(lo * 1024, hi * 1024)
                eng = nc.scalar if n % 2 == 0 else nc.sync
                eng.dma_start(IN_sb[:, s], IN[:, s])
                if n == 1:
                    nc.scalar.dma_start(tri4_sb[:], tri4[:])

            S_prev = stp.tile([128, 128], F16, tag="S")
            nc.gpsimd.memset(S_prev[:], 0.0)

            state = {}   # g -> S_list
            kpst = {}    # g -> KPps (awaiting chain emission)
            amst = {}    # g -> Am4
            otst = {}    # h -> OTps
            for g in range(NGRP + LAG + 1):
                if 1 <= g <= 5:
                    # HAM insurance: no-dep dummy matmuls fill early data
                    # stalls so the PE's activity window never lapses
                    for _ in range((4, 3, 3, 2, 2)[g - 1]):
                        wp = ps_w.tile([128, 512], F32, tag="w")
                        mm(wp[:], wz[:, :128], wz[:], start=True, stop=True)
                if g < NGRP:
                    # KP matmuls for group g.  high_priority pins KPs ahead
                    # of AT/OT in the static PE order - the chain (the
                    # pacer) must never wait on them.
                    KPps = ps_kp.tile([128, 512], F32, tag="kp")
                    with tc.high_priority(offset=100000):
                        for q in range(4):
                            m = 4 * g + q
                            qs = slice(q * 128, (q + 1) * 128)
                            mm(KPps[:, qs], in_sl(IN_sb, m, 0),
                               in_sl(IN_sb, m, 1), start=True, stop=True)
                    kpst[g] = KPps
                ga = g - ATLAG
                if 0 <= ga < NGRP:
                    # AT matmuls trail the KP/chain path (QK data ships later)
                    ATps = ps_at.tile([128, 512], F32, tag="at")
                    for q in range(4):
                        m = 4 * ga + q
                        qs = slice(q * 128, (q + 1) * 128)
                        mm(ATps[:, qs], in_sl(IN_sb, m, 2), in_sl(IN_sb, m, 3),
                           start=True, stop=True)
                    Am4 = amp.tile([128, 512], F16, tag="am")
                    if ga >= NGRP - DVE_MASK:
                        # last group: one DVE op straight from PSUM (its
                        # chain is done by now) - shortest tail path.
                        # (AT * 2^-12) * (4096 * mask) == AT * mask exactly.
                        nc.vector.scalar_tensor_tensor(
                            out=Am4[:], in0=ATps[:], scalar=2.0 ** -12,
                            in1=tri4_sb[:], op0=mybir.AluOpType.mult,
                            op1=mybir.AluOpType.mult)
                    else:
                        # AT evac x2^-12 (ACT), mask via x{4096,0} tri (GPSIMD)
                        ATs = atsp.tile([128, 512], F16, tag="ats")
                        nc.scalar.mul(ATs[:], ATps[:], 2.0 ** -12)
                        nc.gpsimd.tensor_mul(Am4[:], ATs[:], tri4_sb[:])
                    amst[ga] = Am4

                # serial S chain (DVE).  The LAST group is emitted only
                # after both DVE masks above, so in the DVE FIFO the masks
                # run inside the chain's natural KP_7 stall window and Am_7
                # is ready the moment the final STT lands (shorter tail).
                chain_g = None
                if g < NGRP - 1:
                    chain_g = g
                elif g == NGRP - 1 + ATLAG:
                    chain_g = NGRP - 1
                if chain_g is not None:
                    KPps = kpst.pop(chain_g)
                    S_list = [S_prev]
                    for q in range(4):
                        qs = slice(q * 128, (q + 1) * 128)
                        S_new = stp.tile([128, 128], F16, tag="S")
                        nc.vector.scalar_tensor_tensor(
                            out=S_new[:], in0=S_prev[:], scalar=g128_sb[:, 0:1],
                            in1=KPps[:, qs], op0=mybir.AluOpType.mult,
                            op1=mybir.AluOpType.add)
                        S_list.append(S_new)
                        S_prev = S_new
                    state[chain_g] = S_list

                h = g - LAG
                if 0 <= h < NGRP:
                    S_list = state.pop(h)
                    Am4 = amst.pop(h)
                    pool = ps_w if h == NGRP - 1 else ps_ot
                    OTps = pool.tile([128, 512], F32,
                                     tag="w" if h == NGRP - 1 else "ot")
                    for q in range(4):
                        m = 4 * h + q
                        qs = slice(q * 128, (q + 1) * 128)
                        mm(OTps[:, qs], in_sl(IN_sb, m, 1), Am4[:, qs],
                           start=True, stop=False)
                        mm(OTps[:, qs], S_list[q][:], in_sl(IN_sb, m, 3),
                           start=False, stop=True)
                    otst[h] = OTps
                # evac one iteration later so it never blocks the ACT FIFO;
                # the last two groups ride DVE, whose chain is done by then
                h2 = g - LAG - 1
                if h2 >= 0:
                    OTps = otst.pop(h2)
                    gs = slice(h2 * 512, (h2 + 1) * 512)
                    if h2 == NGRP - 1:
                        for half in range(2):
                            hs = slice(h2 * 512 + half * 256,
                                       h2 * 512 + (half + 1) * 256)
                            nc.vector.tensor_copy(otT_sb[:, hs],
                                                  OTps[:, half * 256:
                                                       (half + 1) * 256])
                            nc.sync.dma_start(otT[:, hs], otT_sb[:, hs])
                    else:
                        if h2 == NGRP - 2:
                            nc.vector.tensor_copy(otT_sb[:, gs], OTps[:])
                        else:
                            nc.scalar.copy(otT_sb[:, gs], OTps[:])
                        nc.sync.dma_start(otT[:, gs], otT_sb[:, gs])

    _split_multi_waits(nc)
    _PROG[key] = nc
    return nc


def _host_prep(q_alpha, k, h_norm, gamma_vec, causal_mask):
    gamma = np.clip(np.asarray(gamma_vec, np.float64), 1e-8, None)
    log_g = np.log(gamma)
    i_loc = (np.arange(W) % 128).astype(np.float64)
    Sq = np.exp(np.outer(i_loc - 64, log_g))        # gamma^(i%128-64)
    Skc = np.exp(np.outer(64 - i_loc, log_g))       # gamma^(64-j%128)
    Sk2 = np.exp(np.outer(192 - i_loc, log_g))      # gamma^(192-j%128)
    g128 = np.exp(128 * log_g).astype(np.float32)

    tri = np.asarray(causal_mask, np.float32).T * 4096.0  # [j,i], 4096 if i>=j
    tri4a = np.ascontiguousarray(np.tile(tri, (1, 4)))
    g128a = np.ascontiguousarray(g128.reshape(128, 1))

    def blockify(x):  # [W, 128] -> [128, (blk, 128)]
        return np.ascontiguousarray(
            x.reshape(NBLK, 128, 128).transpose(1, 0, 2).reshape(128, W))

    in_maps = []
    for b in range(B):
        q64 = np.asarray(q_alpha[b], np.float64)
        k64 = np.asarray(k[b], np.float64)
        ksT = (k64 * Skc).T.astype(np.float16)      # [R, W]
        qsT = (q64 * Sq).T.astype(np.float16)
        k2n = blockify((k64 * Sk2).astype(np.float16))
        hn = blockify(np.asarray(h_norm[b], np.float16))
        IN = np.empty((128, 4 * W), np.float16)
        for g in range(NGRP):
            s = slice(g * 512, (g + 1) * 512)
            ckh = KH_POS[g] * 1024
            cqk = QK_POS[g] * 1024
            for q in range(4):
                c0 = ckh + q * 256
                bs = slice(g * 512 + q * 128, g * 512 + (q + 1) * 128)
                IN[:, c0:c0 + 128] = k2n[:, bs]
                IN[:, c0 + 128:c0 + 256] = hn[:, bs]
            IN[:, cqk + 0:cqk + 512] = ksT[:, s]
            IN[:, cqk + 512:cqk + 1024] = qsT[:, s]
        in_maps.append({"IN": IN, "tri4": tri4a, "g128": g128a})
    return in_maps


def _ensure_ntff_hook():
    try:
        from antenv import axon_hooks  # noqa: F401
        return
    except ImportError:
        pass
    import types
    import antenv
    try:
        import trn_agent_boot.trn_boot as tb
        hook = tb._ntff_profile_via_ctypes("/opt/axon/libaxon_pjrt.so")
    except Exception:
        hook = None
    mod = types.ModuleType("antenv.axon_hooks")
    mod.get_axon_ntff_profile_hook = lambda: hook
    mod.set_axon_ntff_profile_hook = lambda h: None
    sys.modules["antenv.axon_hooks"] = mod
    antenv.axon_hooks = mod


_last = {"exec_time_ns": None}


def kernel(q_alpha, k, h_norm, gamma_vec, causal_mask, decay_diff,
           _trace=False):
    trace = _trace or os.environ.get("BD_TRACE", "0") == "1"
    from concourse.bass_utils import run_bass_kernel_spmd

    nc = _build_program()
    in_maps = _host_prep(q_alpha, k, h_norm, gamma_vec, causal_mask)
    kwargs = {}
    if trace:
        _ensure_ntff_hook()
        import concourse.bass_utils as bu
        bu.upload_artifacts = lambda tmpdir: tmpdir  # no bucket in container
        kwargs = dict(trace=True, tmpdir=os.environ.get("BD_TRACE_DIR") or None)
    res = run_bass_kernel_spmd(nc, in_maps, list(range(B)), **kwargs)
    _last["exec_time_ns"] = res.exec_time_ns
    out = np.empty((B, W, D), np.float32)
    for b in range(B):
        out[b] = res.results[b]["otT"].T.astype(np.float32)
    return out
